# revision 46
# baseline (speedup 1.0000x reference)
"""Trainium2 Bass kernel for nn_MoE_4088808866374.

Top-1 MoE (B=4, S=1024, D=1024, E=8, F=2816, K=1) + shared expert.

The reference computes all 8 experts densely over all 4096 tokens, but the
sigmoid gate is exactly 0 for non-top-1 experts (sigmoid(-inf)), and zero
inputs propagate exactly through SwiGLU (silu(0)=0, 0*w=0). So a sparse
dispatch computes the identical result with ~4.5x fewer FLOPs.

Sharding (8 cores, all phases SPMD — one NEFF, per-core data differs):
  - Phase R (routed, 512 slots = exactly 4 y-tiles): core e holds expert
    e's weights and the first 512 tokens routed to expert e (gate-scaled,
    zero-padded if fewer). Host dispatch/combine plays the all-to-all.
  - Phase S (shared, 512 slots): data-parallel shared expert; core e
    processes tokens [512e, 512e+512) with replicated shared weights.
  - Phase V (overflow, tensor-parallel over F): tokens beyond slot 512 of
    hot experts, in <=128-token groups. Every core processes the SAME
    overflow tokens but a DISJOINT 3-of-22 f-chunk slice of that expert's
    weights (host packs each core's slice; zero-padded on the last core).
    Partial [V, D] outputs are summed on the host. This balances any
    routing skew with no per-core program differences and only ~1/8 of
    the hot experts' weights re-streamed per core.
  - Router (4096x1024x8 matmul + top-1 + sigmoid = 0.05% of total FLOPs)
    runs host-side since it determines the dispatch itself.

All matmuls in bf16 (1 cyc/row at any moving size, fp32 PSUM accumulate,
~4.4e-3 rel err vs the 2e-2 gate), which also halves HBM traffic vs f32r.
Phase V is weight-streaming-bound, so its shapes are chosen to hit the
128-row stationary-load bound exactly: h-matmuls move over 128-padded
token slots; y-matmuls move over d (512 wide) with real-token stationary
operands. Main-phase y keeps v1's measured-fastest form (mid stationary,
w2 moving in 352/352/320 d-slices, w2 resident, loaded during h).
"""

import numpy as np

import concourse.bacc as bacc
import concourse.mybir as mybir
import concourse.tile as tile
from concourse import bass_utils

# Problem constants (hardcoded per harness contract).
B, S, D, E, F = 4, 1024, 1024, 8, 2816
A = B * S            # 4096 tokens
R = 512              # routed slots per core (phase R)
T = 512              # shared tokens per core (phase S)
P = 128
D_CH = D // P        # 8
F_CH = F // P        # 22
FS = 3               # f-chunks per core in phase V (ceil(22/8))

_BUILD_CACHE = {}

H_CHUNK = 512     # h-phase token moving chunk (psum bank = 512 fp32)


def _build(groups: tuple, reps: int = 1):
    """Build + compile the SPMD Bass kernel.

    groups: static sizes of the per-expert overflow token groups handled by
    the F-tensor-parallel phase V (empty tuple = no overflow phase).
    reps>1 wraps the body in a hardware For_i loop (used by the test harness
    to measure per-execution device time as a slope, amortizing the ~100ms
    axon dispatch overhead)."""
    key = (groups, reps)
    if key in _BUILD_CACHE:
        return _BUILD_CACHE[key]

    bdt = mybir.dt.bfloat16
    fp32 = mybir.dt.float32
    G = len(groups)
    V = int(sum(groups))

    nc = bacc.Bacc("TRN2", target_bir_lowering=False, debug=False)

    # DRAM I/O (per core). Weight layouts are host-packed so every DMA is
    # contiguous per partition:
    #   w1/w3: [P(d_inner), F_CH, D_CH, P(f_inner)]   h-slab = [:, fc]
    #   w2:    [P(f_inner), F_CH, D]                  resident, [:, fc] loads
    #   x:     [P(d_inner), D_CH, ntok]
    xr = nc.dram_tensor("xr", [P, D_CH, R], bdt, kind="ExternalInput")
    xs = nc.dram_tensor("xs", [P, D_CH, T], bdt, kind="ExternalInput")
    w1r = nc.dram_tensor("w1r", [P, F_CH, D_CH, P], bdt, kind="ExternalInput")
    w3r = nc.dram_tensor("w3r", [P, F_CH, D_CH, P], bdt, kind="ExternalInput")
    w2r = nc.dram_tensor("w2r", [P, F_CH, D], bdt, kind="ExternalInput")
    w1s = nc.dram_tensor("w1s", [P, F_CH, D_CH, P], bdt, kind="ExternalInput")
    w3s = nc.dram_tensor("w3s", [P, F_CH, D_CH, P], bdt, kind="ExternalInput")
    w2s = nc.dram_tensor("w2s", [P, F_CH, D], bdt, kind="ExternalInput")
    yr = nc.dram_tensor("yr", [R, D], bdt, kind="ExternalOutput")
    ys = nc.dram_tensor("ys", [T, D], bdt, kind="ExternalOutput")
    if V:
        # xv token groups are host-padded to 128 slots each so every V
        # matmul's moving dim >= the 128-row stationary load (no PE stalls)
        VP = G * P
        xv = nc.dram_tensor("xv", [P, D_CH, VP], bdt, kind="ExternalInput")
        w1v = nc.dram_tensor("w1v", [P, G, FS, D_CH, P], bdt,
                             kind="ExternalInput")
        w3v = nc.dram_tensor("w3v", [P, G, FS, D_CH, P], bdt,
                             kind="ExternalInput")
        w2v = nc.dram_tensor("w2v", [P, G, FS, D], bdt,
                             kind="ExternalInput")
        yv = nc.dram_tensor("yv", [V, D], fp32, kind="ExternalOutput")
    # tiny pass-through token so the test harness can chain executions
    # back-to-back (data dependency defeats CSE / enforces ordering)
    tok = nc.dram_tensor("tok", [1, 1], fp32, kind="ExternalInput")
    tokout = nc.dram_tensor("tokout", [1, 1], fp32, kind="ExternalOutput")

    with tile.TileContext(nc) as tc:
        with tc.tile_pool(name="xpool", bufs=3) as xpool, \
             tc.tile_pool(name="wpool", bufs=8) as wpool, \
             tc.tile_pool(name="w2pool", bufs=1) as w2pool, \
             tc.tile_pool(name="midpool", bufs=2) as midpool, \
             tc.tile_pool(name="vpool", bufs=1) as vpool, \
             tc.tile_pool(name="vslab", bufs=min(3 * max(G, 1), 12)) as vslab, \
             tc.tile_pool(name="tmp", bufs=2) as tmp, \
             tc.tile_pool(name="ytmp", bufs=3) as ytmp, \
             tc.tile_pool(name="psA", bufs=2, space="PSUM") as psA, \
             tc.tile_pool(name="psB", bufs=2, space="PSUM") as psB, \
             tc.tile_pool(name="psY", bufs=3, space="PSUM") as psY:

            def swiglu(xT_d, w1_d, w3_d, w2_d, y_d, ntok, phase):
                hchunks = [(o, min(H_CHUNK, ntok - o))
                           for o in range(0, ntok, H_CHUNK)]
                # activations resident; split the load per d-chunk so the
                # first matmul only waits for its own slice
                xT_sb = xpool.tile([P, D_CH, ntok], bdt, tag="x",
                                   name=f"x_{phase}")
                for d in range(D_CH):
                    nc.scalar.dma_start(xT_sb[:, d], xT_d.ap()[:, d])
                # w2 resident; slabs are prefetched inside the h-loop (they
                # are only needed by the y-phase - loading them up front
                # would queue the whole 5.6MB of DMA ahead of the w1 slabs)
                w2_sb = w2pool.tile([P, F_CH, D], bdt, tag="w2res",
                                    name=f"w2_{phase}")
                # mid resident [P(f_inner), F_CH, ntok] bf16
                mid_sb = midpool.tile([P, F_CH, ntok], bdt, tag="mid",
                                      name=f"mid_{phase}")

                # ---- h-phase: mid[f, t] = silu(h1) * h3 ----
                for fc in range(F_CH):
                    w1_sb = wpool.tile([P, D_CH, P], bdt, tag="w1slab",
                                       name=f"w1s_{phase}_{fc}")
                    nc.sync.dma_start(w1_sb[:], w1_d.ap()[:, fc])
                    w3_sb = wpool.tile([P, D_CH, P], bdt, tag="w3slab",
                                       name=f"w3s_{phase}_{fc}")
                    nc.sync.dma_start(w3_sb[:], w3_d.ap()[:, fc])
                    nc.sync.dma_start(w2_sb[:, fc], w2_d.ap()[:, fc])
                    for o, tn in hchunks:
                        ps1 = psA.tile([P, H_CHUNK], fp32, tag="ps1",
                                       name=f"ps1_{phase}_{fc}_{o}")[:, :tn]
                        for d in range(D_CH):
                            nc.tensor.matmul(
                                ps1, w1_sb[:, d], xT_sb[:, d, o:o + tn],
                                start=(d == 0), stop=(d == D_CH - 1))
                        ps3 = psB.tile([P, H_CHUNK], fp32, tag="ps3",
                                       name=f"ps3_{phase}_{fc}_{o}")[:, :tn]
                        for d in range(D_CH):
                            nc.tensor.matmul(
                                ps3, w3_sb[:, d], xT_sb[:, d, o:o + tn],
                                start=(d == 0), stop=(d == D_CH - 1))
                        silu_sb = tmp.tile([P, H_CHUNK], fp32, tag="silu",
                                           name=f"silu_{phase}_{fc}_{o}")[:, :tn]
                        nc.scalar.activation(
                            silu_sb, ps1, mybir.ActivationFunctionType.Silu)
                        nc.vector.tensor_tensor(mid_sb[:, fc, o:o + tn],
                                                silu_sb, ps3,
                                                mybir.AluOpType.mult)

                # ---- y-phase: y[t, d] = sum_f mid[f, t] * w2[f, d] ----
                # d-slices of 352/320: N~320-352 measured ~5% faster
                # per column than N=512 on the PE (w2-load leveling into the
                # y-phase and 4x256 slices were both A/B'd: neutral within
                # noise, so this keeps the validated form)
                D_SLICES = [(0, 352), (352, 352), (704, 320)]
                for ds_, (d0, dn) in enumerate(D_SLICES):
                    for tt in range(ntok // P):
                        psy = psY.tile([P, 512], fp32, tag="psy",
                                       name=f"psy_{phase}_{tt}_{ds_}")[:, :dn]
                        for fc in range(F_CH):
                            nc.tensor.matmul(
                                psy, mid_sb[:, fc, tt * P:(tt + 1) * P],
                                w2_sb[:, fc, d0:d0 + dn],
                                start=(fc == 0), stop=(fc == F_CH - 1))
                        y_sb = ytmp.tile([P, 512], bdt, tag="ysb",
                                         name=f"y_{phase}_{tt}_{ds_}")[:, :dn]
                        nc.scalar.copy(y_sb, psy)
                        nc.scalar.dma_start(
                            y_d.ap()[tt * P:(tt + 1) * P, d0:d0 + dn], y_sb)

            def overflow_phase():
                # Phase V: every core runs the same token groups against its
                # own FS-chunk slice of the hot experts' weights; host sums
                # the 8 partial outputs. Token slots padded to 128/group so
                # h-matmul moving dim covers the 128-row stationary load;
                # y-matmuls move over d (512 wide) with real-token stationary.
                VP = G * P
                xT_sb = xpool.tile([P, D_CH, VP], bdt, tag="x", name="x_v")
                for d in range(D_CH):
                    nc.scalar.dma_start(xT_sb[:, d], xv.ap()[:, d])
                mid_sb = vpool.tile([P, G, FS, P], bdt, tag="midv",
                                    name="mid_v")
                # issue the y-phase w2v slab loads ahead of the h slabs:
                # they are independent, and queued last they would race
                # V.y's first matmuls (~2.2MB against a ~10us window)
                w2_sbs = {}
                for g in range(G):
                    for vfc in range(FS):
                        w2_sb = vslab.tile([P, D], bdt, tag="w2vslab",
                                           name=f"w2v_{g}_{vfc}")
                        nc.sync.dma_start(w2_sb[:], w2v.ap()[:, g, vfc])
                        w2_sbs[(g, vfc)] = w2_sb
                for g in range(G):
                    for vfc in range(FS):
                        w1_sb = wpool.tile([P, D_CH, P], bdt, tag="w1slab",
                                           name=f"w1v_{g}_{vfc}")
                        nc.sync.dma_start(w1_sb[:], w1v.ap()[:, g, vfc])
                        w3_sb = wpool.tile([P, D_CH, P], bdt, tag="w3slab",
                                           name=f"w3v_{g}_{vfc}")
                        nc.sync.dma_start(w3_sb[:], w3v.ap()[:, g, vfc])
                        ps1 = psA.tile([P, P], fp32, tag="ps1",
                                       name=f"ps1_v_{g}_{vfc}")
                        for d in range(D_CH):
                            nc.tensor.matmul(
                                ps1, w1_sb[:, d],
                                xT_sb[:, d, g * P:(g + 1) * P],
                                start=(d == 0), stop=(d == D_CH - 1))
                        ps3 = psB.tile([P, P], fp32, tag="ps3",
                                       name=f"ps3_v_{g}_{vfc}")
                        for d in range(D_CH):
                            nc.tensor.matmul(
                                ps3, w3_sb[:, d],
                                xT_sb[:, d, g * P:(g + 1) * P],
                                start=(d == 0), stop=(d == D_CH - 1))
                        silu_sb = tmp.tile([P, P], fp32, tag="silu",
                                           name=f"silu_v_{g}_{vfc}")
                        nc.scalar.activation(
                            silu_sb, ps1, mybir.ActivationFunctionType.Silu)
                        nc.vector.tensor_tensor(mid_sb[:, g, vfc],
                                                silu_sb, ps3,
                                                mybir.AluOpType.mult)

                o = 0
                for g, tg in enumerate(groups):
                    for dh in range(0, D, 512):
                        psy = psY.tile([P, 512], fp32, tag="psy",
                                       name=f"psy_v_{g}_{dh}")[:tg]
                        for vfc in range(FS):
                            nc.tensor.matmul(
                                psy, mid_sb[:, g, vfc, :tg],
                                w2_sbs[(g, vfc)][:, dh:dh + 512],
                                start=(vfc == 0), stop=(vfc == FS - 1))
                        y_sb = ytmp.tile([P, 512], fp32, tag="yvsb",
                                         name=f"yv_{g}_{dh}")[:tg]
                        nc.scalar.copy(y_sb, psy)
                        nc.scalar.dma_start(
                            yv.ap()[o:o + tg, dh:dh + 512], y_sb)
                    o += tg

            def body():
                swiglu(xr, w1r, w3r, w2r, yr, R, "r")
                swiglu(xs, w1s, w3s, w2s, ys, T, "s")
                if V:
                    overflow_phase()

            if reps == 1:
                body()
            else:
                # staggered_reset avoids the ~2us all-engine barrier per
                # back-edge so the measured slope tracks single-shot time
                with tc.For_i(0, reps, 1, staggered_reset=True):
                    body()
            nc.sync.dma_start(tokout.ap(), tok.ap())

    nc.compile()
    _BUILD_CACHE[key] = nc
    return nc


def _sigmoid32(x):
    x = x.astype(np.float32)
    return np.where(x >= 0, 1.0 / (1.0 + np.exp(-x)),
                    np.exp(x) / (1.0 + np.exp(x))).astype(np.float32)


def _np_bf16():
    import ml_dtypes
    return ml_dtypes.bfloat16


def _pack_w_df(w, np_dt):
    # [D, F] -> [P(d_inner), F_CH, D_CH, P(f_inner)]
    return np.ascontiguousarray(
        w.reshape(D_CH, P, F_CH, P).transpose(1, 2, 0, 3).astype(np_dt))


def _pack_w_fd_res(w, np_dt):
    # [F, D] -> [P(f_inner), F_CH, D]  (resident w2 for the main phases)
    return np.ascontiguousarray(
        w.reshape(F_CH, P, D).transpose(1, 0, 2).astype(np_dt))


def _pack_xT(x, np_dt):
    # [n, D] -> [P(d_inner), D_CH, n]
    return np.ascontiguousarray(
        x.reshape(-1, D_CH, P).transpose(2, 1, 0).astype(np_dt))


def prepare(x_bsD, router_DE, w1_eDF, w3_eDF, w2_eFD, ws1_DF, ws3_DF, ws2_FD):
    """Host-side routing + dispatch. Returns (in_maps, aux) for the SPMD run."""
    np_dt = _np_bf16()

    x = np.ascontiguousarray(np.asarray(x_bsD, np.float32).reshape(A, D))
    scores = x @ np.asarray(router_DE, np.float32)          # [A, E]
    top1 = np.argmax(scores, axis=1)                        # [A]
    gate = _sigmoid32(scores[np.arange(A), top1])           # [A]

    idx_e = [np.nonzero(top1 == e)[0] for e in range(E)]
    counts = np.array([len(i) for i in idx_e])

    # overflow groups: tokens beyond slot R of each hot expert, handled
    # F-tensor-parallel in phase V; split into <=128-token groups (the V
    # h-phase processes one 128-slot block per group)
    ov = []
    for e in range(E):
        c = int(counts[e])
        for lo in range(R, c, P):
            ov.append((e, lo, min(P, c - lo)))
    groups = tuple(tg for _, _, tg in ov)
    V = int(sum(groups))

    xg = gate[:, None] * x                                   # gate-scaled
    w1sp = _pack_w_df(np.asarray(ws1_DF, np.float32), np_dt)
    w3sp = _pack_w_df(np.asarray(ws3_DF, np.float32), np_dt)
    w2sp = _pack_w_fd_res(np.asarray(ws2_FD, np.float32), np_dt)

    # phase-V inputs: identical token buffer on every core; per-core weight
    # slices of FS f-chunks (zero-padded past chunk F_CH-1)
    if V:
        vx = np.zeros((len(groups) * P, D), np.float32)
        for g, (e, lo, tg) in enumerate(ov):
            vx[g * P:g * P + tg] = xg[idx_e[e][lo:lo + tg]]
        xvp = _pack_xT(vx, np_dt)
        packs = {}
        for e in {e for e, _, _ in ov}:
            packs[e] = (
                _pack_w_df(np.asarray(w1_eDF[e], np.float32), np_dt),
                _pack_w_df(np.asarray(w3_eDF[e], np.float32), np_dt),
                _pack_w_fd_res(np.asarray(w2_eFD[e], np.float32), np_dt))

    in_maps = []
    for c in range(E):
        xr_ = np.zeros((R, D), np.float32)
        n = min(int(counts[c]), R)
        xr_[:n] = xg[idx_e[c][:n]]
        m = {
            "xr": _pack_xT(xr_, np_dt),
            "xs": _pack_xT(x[c * T:(c + 1) * T], np_dt),
            "w1r": _pack_w_df(np.asarray(w1_eDF[c], np.float32), np_dt),
            "w3r": _pack_w_df(np.asarray(w3_eDF[c], np.float32), np_dt),
            "w2r": _pack_w_fd_res(np.asarray(w2_eFD[c], np.float32), np_dt),
            "w1s": w1sp, "w3s": w3sp, "w2s": w2sp,
            "tok": np.zeros((1, 1), np.float32),
        }
        if V:
            f0 = c * FS
            w1v = np.zeros((P, len(groups), FS, D_CH, P), np_dt)
            w3v = np.zeros_like(w1v)
            w2v = np.zeros((P, len(groups), FS, D), np_dt)
            nf = max(0, min(FS, F_CH - f0))
            for g, (e, lo, tg) in enumerate(ov):
                if nf > 0:
                    # w1 packed [P, F_CH, D_CH, P]: take f-chunks f0:f0+nf
                    w1v[:, g, :nf] = packs[e][0][:, f0:f0 + nf]
                    w3v[:, g, :nf] = packs[e][1][:, f0:f0 + nf]
                    # w2 packed [P(f_in), F_CH, D]: take f-chunks f0:f0+nf
                    w2v[:, g, :nf] = packs[e][2][:, f0:f0 + nf]
            m["xv"] = xvp
            m["w1v"] = w1v
            m["w3v"] = w3v
            m["w2v"] = w2v
        in_maps.append(m)
    return in_maps, (idx_e, counts, groups, ov)


def combine(results, aux):
    """Merge per-core outputs into the full [B, S, D] output."""
    idx_e, counts, groups, ov = aux
    out = np.empty((A, D), np.float32)
    for c in range(E):
        out[c * T:(c + 1) * T] = np.asarray(results[c]["ys"], np.float32)
    for c in range(E):
        n = min(int(counts[c]), R)
        out[idx_e[c][:n]] += np.asarray(results[c]["yr"], np.float32)[:n]
    if groups:
        yv = np.zeros((int(sum(groups)), D), np.float32)
        for c in range(E):
            yv += np.asarray(results[c]["yv"], np.float32)
        o = 0
        for e, lo, tg in ov:
            out[idx_e[e][lo:lo + tg]] += yv[o:o + tg]
            o += tg
    return out.reshape(B, S, D)


def kernel(x_bsD, router_DE, w1_eDF, w3_eDF, w2_eFD, ws1_DF, ws3_DF, ws2_FD):
    in_maps, aux = prepare(x_bsD, router_DE, w1_eDF, w3_eDF, w2_eFD,
                           ws1_DF, ws3_DF, ws2_FD)
    nc = _build(aux[2])
    res = bass_utils.run_bass_kernel_spmd(nc, in_maps, core_ids=list(range(E)))
    return combine(res.results, aux)
